# revision 11
# baseline (speedup 1.0000x reference)
"""BigBird simulated attention on 8 Trainium2 NeuronCores.

Strategy
--------
B*H = 24 (batch, head) pairs are sharded 3-per-core across 8 cores (data/head
parallel). The BigBird mask is block-constant on 64x64 tiles, so the host
compresses it to a 64x64 block map and bakes a block-sparse schedule directly
into the instruction stream (the mask never goes to the device).

Per (head, q-block of 64 rows) scores are computed TRANSPOSED (S^T: k on
partitions, q on free):

  S^T[k, q] = sum_d K[k, d] Q[q, d]    (lhsT = K^T block cols, rhs = Q^T)
  P^T = exp(S^T / 8)                    (one ScalarE activation per wave)

PV runs with the V-pair as the STATIONARY operand and P^T as the moving one:

  acc^T[:, q] += Vaug_pair^T @ P^T_pair[:, q]   with Vaug = [V | 1]

acc^T is [65, q]: row 64 is the softmax denominator (ones column of Vaug).
The division happens on the HOST after the un-normalized [65, q] tiles are
DMA'd back (bf16) -- softmax is shift-invariant and exp can't overflow
(scores ~N(0,1) after the 1/8 scale), so no max-subtraction is needed.

Performance model (measured via microbenchmarks on this toolchain):
  - The PE runs at a 1.2 GHz mid p-state until it has executed ~3-5us with
    NO idle gaps, then doubles to 2.4 GHz. Holding the ramp is worth 2x on
    everything, so the schedule is built around a gap-free PE stream:
      * PV for wave w is emitted 2 waves behind QK (PV_LAG) so the exp
        activation is never on the PE's critical path,
      * per-wave vp tiles are prefetched ~9 waves ahead,
      * output PSUM banks are opened by the G-unit PV itself (start=True
        marks the whole 2KB bank zero-pending; later PVs first-touch
        overwrite) -- no dummy matmuls, no per-bank PE stall,
      * the wave stream runs continuously across all 3 heads.
  - LDWEIGHTS overlaps MATMUL (fast weight load); per-matmul pitch at
    2.4 GHz is ~34ns for 64-col moving, so matmul COUNT x width dominates.
  - K-pair stationaries for scattered (rand) blocks are 3-dim strided APs
    straight into the resident kT tile -- no gathered kp stream from HBM.
    Gathered V pairs (vp) still stream, since the two V blocks must land on
    opposite partition halves, which an affine AP cannot do.

Output PSUM: 8 banks x 8 q-positions (512 f32 cols = exactly one bank).
Bank position p holds q-block PERM[p], PERM = [1..62, 0, 63]; rows 0/63
(the dense global rows) share bank 7 with middle rows 57..62's positions.

Sync: the Tile framework tracks all deps; after emission the Bacc passes
move_matmul_waits_to_ldweights + generate_event_semaphores re-establish the
TRN2 "at most one sync wait per instruction" constraint.
"""

import numpy as np

import concourse.bass as bass
import concourse.tile as tile
from concourse import mybir
from concourse.bass_utils import run_bass_kernel_spmd

B, H, S, D = 2, 12, 4096, 64
BLK = 64
NB = S // BLK            # 64 blocks per axis
DA = D + 1               # v plus ones column
NCORES = 8
HPC = B * H // NCORES    # heads per core
SCALE = 1.0 / 8.0        # 1/sqrt(64)
WAVE_CHUNKS = 8          # 8*64 f32 cols = exactly 1 PSUM bank per score tile
NCHUNK = S // 128        # natural 128-row chunks of V
PAIR_G = NCHUNK          # resident pair index for the global (0, 63) pair
NPAIR = NCHUNK + 1
PV_LAG = 3               # waves PV trails QK by
VP_LOOKAHEAD = 16        # waves of vp prefetch
GW = 4                   # waves per gather-DMA group
VP_BUFS = 7              # 4-wave group tiles

# q-block order on device: middle rows first, then the two full rows
PERM = list(range(1, NB - 1)) + [0, NB - 1]
QPOS = {r: p for p, r in enumerate(PERM)}
# output banks: 8 banks x 8 positions (one full PSUM bank each)
BANKS = [range(8 * b, 8 * b + 8) for b in range(8)]
NBANK = len(BANKS)
POSBANK = {p: b for b, rng in enumerate(BANKS) for p in rng}

F32 = mybir.dt.float32
BF16 = mybir.dt.bfloat16


# ----------------------------------------------------------------- schedule

def _block_mask(mask: np.ndarray) -> np.ndarray:
    m = np.asarray(mask).reshape(NB, BLK, NB, BLK)
    bm = m[:, 0, :, 0]
    assert bool(np.all(m == bm[:, None, :, None])), (
        "mask is not 64x64 block-constant; this kernel's schedule requires it"
    )
    return bm > 0


def _row_chunks(bm: np.ndarray, i: int):
    L = set(np.nonzero(bm[i])[0].tolist())
    full = len(L) == NB
    has_g = False
    if not full and 0 in L and NB - 1 in L:
        L -= {0, NB - 1}
        has_g = True
    aligned = [t for t in range(NB // 2) if 2 * t in L and 2 * t + 1 in L]
    cov = {b for t in aligned for b in (2 * t, 2 * t + 1)}
    singles = sorted(L - cov)
    spairs = [(singles[k], singles[k + 1] if k + 1 < len(singles) else None)
              for k in range(0, len(singles), 2)]
    return full, has_g, aligned, spairs


def _usize(u):
    if u[0] == "G":
        return u[2]
    return 2 if u[0] in ("W", "FR") else 1


def _ubank(u):
    if u[0] == "G":
        return POSBANK[u[1]]
    if u[0] == "W":
        return POSBANK[u[2]]
    if u[0] == "FR":
        return POSBANK[NB - 2]
    return POSBANK[u[1]]


def _touched_banks(u):
    if u[0] == "G":
        return {POSBANK[u[1]]}
    if u[0] == "W":
        return {POSBANK[u[2]], POSBANK[u[2] + 1]}
    if u[0] == "FR":
        return {POSBANK[NB - 2]}
    return {POSBANK[u[1]]}


def _build_units(bm: np.ndarray):
    info = {i: _row_chunks(bm, i) for i in range(NB)}
    wset = {}
    for t in range(NB // 2):
        r0, r1 = 2 * t, 2 * t + 1
        if (not info[r0][0] and not info[r1][0]
                and t in info[r0][2] and t in info[r1][2]):
            wset[t] = (r0, r1)
    units = []
    for b, prange in enumerate(BANKS):
        gpos = [p for p in prange if p < NB - 2]
        if gpos:
            assert all(info[PERM[p]][1] for p in gpos)
            units.append(("G", gpos[0], len(gpos)))
        # resident-pair units first, streamed (vp) S units last, so the
        # head's opening waves don't stall on the gather DMAs
        sunits = []
        for p in prange:
            if p >= NB - 2:
                continue
            r = PERM[p]
            full, has_g, aligned, spairs = info[r]
            assert not full
            for t in aligned:
                if t in wset and r in wset[t]:
                    if r == wset[t][0]:
                        units.append(("W", t, QPOS[wset[t][0]]))
                else:
                    units.append(("P", p, t))
            for (gA, gB) in spairs:
                sunits.append(("S", p, gA, gB))
        units.extend(sunits)
        if b == NBANK - 1:
            for t in range(NB // 2):
                units.append(("FR", t))
    return units


def _pack(units):
    """Pack units into 16-slot waves; multi-slot units must not cross an
    8-slot PSUM score-bank boundary. A unit may only be placed once the
    G unit of every output bank it touches is placed (the G opens the bank
    with start=True; any earlier touch would corrupt the accumulation).
    Lookahead picks are restricted to the head unit's bank or the next, so
    at most two output banks are ever accumulating at once."""
    pending = list(units)
    flat = []
    pos = 0
    opened = set()
    while pending:
        rem = 8 - (pos % 8)
        head_bank = _ubank(pending[0])
        pick = None
        for idx in range(min(len(pending), 24)):
            u = pending[idx]
            if _usize(u) > rem or _ubank(u) > head_bank + 1:
                continue
            if u[0] == "G":
                pick = idx
                break
            if all(b in opened for b in _touched_banks(u)):
                pick = idx
                break
        if pick is None:
            flat.append((pos, ("X",)))
            pos += 1
        else:
            u = pending.pop(pick)
            if u[0] == "G":
                opened.add(POSBANK[u[1]])
            flat.append((pos, u))
            pos += _usize(u)
    waves = []
    for (p, u) in flat:
        w = p // WAVE_CHUNKS
        while len(waves) <= w:
            waves.append([])
        waves[w].append((p % WAVE_CHUNKS, u))
    return waves


def _unit_pv(u, slot):
    """PV matmuls for a unit: (pos0, pT slot0, width, source).

    source: ("v2", pair) resident, or ("vp", sidx) gathered."""
    k = u[0]
    if k == "G":
        return [(u[1], slot, u[2], ("v2", PAIR_G))]
    if k == "FR":
        return [(NB - 2, slot, 2, ("v2", u[1]))]
    if k == "W":
        p0 = u[2]
        if POSBANK[p0] == POSBANK[p0 + 1]:
            return [(p0, slot, 2, ("v2", u[1]))]
        return [(p0, slot, 1, ("v2", u[1])),
                (p0 + 1, slot + 1, 1, ("v2", u[1]))]
    if k == "P":
        return [(u[1], slot, 1, ("v2", u[2]))]
    return [(u[1], slot, 1, ("vp", u[4]))]


def _build_schedule(bm: np.ndarray):
    units = _build_units(bm)
    waves = _pack(units)
    ns = 0
    waves2 = []
    pos_chunks = np.zeros(NB, dtype=np.int64)
    for wave in waves:
        w2 = []
        for slot, u in wave:
            if u[0] == "S":
                u = u + (ns,)
                ns += 1
            w2.append((slot, u))
            if u[0] != "X":
                for (p0, s0, width, src) in _unit_pv(u, slot):
                    for j in range(width):
                        pos_chunks[p0 + j] += 1
        waves2.append(w2)
    return waves2, ns, pos_chunks


# ------------------------------------------------------------------ program

def _build_program(bm: np.ndarray):
    import os as _os
    hpc = int(_os.environ.get("BB_HPC", HPC))
    waves, ns, pos_chunks = _build_schedule(bm)
    W = len(waves)
    nc = bass.Bass("TRN2", target_bir_lowering=False, debug=False,
                   enable_asserts=False)
    qT_d = nc.dram_tensor("qT", [HPC, 128, S], BF16,
                          kind="ExternalInput")
    kT_d = nc.dram_tensor("kT", [HPC, 64, S + 2 * BLK], BF16,
                          kind="ExternalInput")
    v2_d = nc.dram_tensor("v2", [HPC, 128, NPAIR * DA], BF16,
                          kind="ExternalInput")
    vp_d = nc.dram_tensor("vp", [HPC, 128, max(ns, 1) * DA], BF16,
                          kind="ExternalInput")
    kp_d = nc.dram_tensor("kp", [HPC, 64, max(ns, 1) * 128], BF16,
                          kind="ExternalInput")
    o_d = [nc.dram_tensor(f"o_{hh}", [NBANK, DA, 8 * BLK], BF16,
                          kind="ExternalOutput") for hh in range(HPC)]

    with tile.TileContext(nc) as tc:
        with (
            tc.tile_pool(name="wq", bufs=HPC) as wq,
            tc.tile_pool(name="wk", bufs=HPC) as wk,
            tc.tile_pool(name="wv", bufs=HPC) as wv,
            tc.tile_pool(name="vpp", bufs=VP_BUFS) as vppool,
            tc.tile_pool(name="kpp", bufs=VP_BUFS) as kppool,
            tc.tile_pool(name="pT", bufs=PV_LAG + 2) as ppool,
            tc.tile_pool(name="st", bufs=4, space="PSUM") as stpool,
            tc.tile_pool(name="ob", bufs=4, space="PSUM") as obpool,
            tc.tile_pool(name="fin", bufs=3) as fpool,
        ):
            tiles = {}

            def load_head(h, part=None):
                # contraction is zero-padded to 128: qT rows 64..127 are
                # zeros (host-baked) so kT/kp rows 64..127 may hold any
                # defined bf16 (their products are multiplied by zero)
                if part == 2:
                    qT, kT, v2 = tiles[h]
                    for a, b in ((1152, 2560), (2560, S)):
                        nc.gpsimd.dma_start(out=qT[:, a:b],
                                            in_=qT_d[h][:, a:b])
                        nc.sync.dma_start(out=kT[0:64, a:b],
                                          in_=kT_d[h][:, a:b])
                        nc.gpsimd.dma_start(out=kT[64:128, a:b],
                                            in_=kT_d[h][:, a:b])
                    nc.scalar.dma_start(out=v2[:, 650: 32 * DA],
                                        in_=v2_d[h][:, 650: 32 * DA])
                    return
                qT = wq.tile([128, S], BF16, tag="qT", name=f"qT{h}")
                kT = wk.tile([128, S + 2 * BLK], BF16, tag="kT",
                             name=f"kT{h}")
                v2 = wv.tile([128, NPAIR * DA], BF16, tag="v2",
                             name=f"v2_{h}")
                tiles[h] = (qT, kT, v2)
                if part == 1:
                    # head 0 runway: just what waves 0..~4 touch -- q
                    # positions 0..17, k blocks 0..17, the global K pair,
                    # V pairs 0..9 and the global V pair
                    nc.gpsimd.dma_start(out=qT[:, 0:1152],
                                        in_=qT_d[h][:, 0:1152])
                    nc.sync.dma_start(out=kT[0:64, 0:1152],
                                      in_=kT_d[h][:, 0:1152])
                    nc.sync.dma_start(out=kT[0:64, S:],
                                      in_=kT_d[h][:, S:])
                    nc.gpsimd.dma_start(out=kT[64:128, 0:1152],
                                        in_=kT_d[h][:, 0:1152])
                    nc.gpsimd.dma_start(out=kT[64:128, S:],
                                        in_=kT_d[h][:, S:])
                    nc.scalar.dma_start(out=v2[:, 0:650],
                                        in_=v2_d[h][:, 0:650])
                    nc.scalar.dma_start(out=v2[:, 32 * DA:],
                                        in_=v2_d[h][:, 32 * DA:])
                    return
                nc.gpsimd.dma_start(out=qT, in_=qT_d[h])
                nc.sync.dma_start(out=kT[0:64, :], in_=kT_d[h])
                nc.gpsimd.dma_start(out=kT[64:128, :], in_=kT_d[h])
                nc.scalar.dma_start(out=v2, in_=v2_d[h])

            vps = {}
            kps = {}
            pair_s0 = {}
            kp_seed = [0]

            def load_gather(gpair):
                """One vp + one kp DMA covering a GW-wave group."""
                h, wp = divmod(gpair, (W + GW - 1) // GW)
                if h >= hpc:
                    return
                sidxs = []
                for w in range(GW * wp, min(GW * wp + GW, W)):
                    sidxs += [u[4] for _, u in waves[w] if u[0] == "S"]
                if not sidxs:
                    return
                s0, scnt = min(sidxs), len(sidxs)
                assert sidxs == list(range(s0, s0 + scnt))
                vt = vppool.tile([128, GW * WAVE_CHUNKS * DA], BF16,
                                 tag="vp", name=f"vph{h}_{wp}")
                kt = kppool.tile([128, GW * WAVE_CHUNKS * 128], BF16,
                                 tag="kp", name=f"kph{h}_{wp}")
                vps[gpair] = vt
                kps[gpair] = kt
                pair_s0[gpair] = s0
                nc.sync.dma_start(
                    out=vt[:, : scnt * DA],
                    in_=vp_d[h][:, s0 * DA: (s0 + scnt) * DA])
                nc.sync.dma_start(
                    out=kt[0:64, : scnt * 128],
                    in_=kp_d[h][:, s0 * 128: (s0 + scnt) * 128])
                # rows 64..127 only need DEFINED bf16 (multiplied by qT's
                # zero rows); uninitialized SBUF can hold NaN patterns, so
                # mirror the lower half SBUF->SBUF (no HBM traffic). On the
                # gpsimd queue: the RAW wait on the lower-half load must not
                # block the SP queue's later gather issues.
                nc.gpsimd.dma_start(
                    out=kt[64:128, : scnt * 128],
                    in_=kt[0:64, : scnt * 128])

            def emit_qk(h, wave, st, kp, k0):
                qT, kT, _ = tiles[h]
                for slot, u in wave:
                    c0 = slot * BLK
                    kind = u[0]
                    if kind == "X":
                        continue
                    if kind == "G":
                        p0, g = u[1], u[2]
                        nc.tensor.matmul(
                            st[:, c0: c0 + g * BLK],
                            lhsT=kT[:, S: S + 2 * BLK],
                            rhs=qT[:, p0 * BLK: (p0 + g) * BLK],
                            start=True, stop=True, skip_group_check=True)
                    elif kind in ("W", "FR"):
                        t = u[1]
                        p0 = u[2] if kind == "W" else NB - 2
                        nc.tensor.matmul(
                            st[:, c0: c0 + 2 * BLK],
                            lhsT=kT[:, 2 * t * BLK: (2 * t + 2) * BLK],
                            rhs=qT[:, p0 * BLK: (p0 + 2) * BLK],
                            start=True, stop=True, skip_group_check=True)
                    elif kind == "P":
                        p, t = u[1], u[2]
                        nc.tensor.matmul(
                            st[:, c0: c0 + BLK],
                            lhsT=kT[:, 2 * t * BLK: (2 * t + 2) * BLK],
                            rhs=qT[:, p * BLK: (p + 1) * BLK],
                            start=True, stop=True, skip_group_check=True)
                    else:  # S: gathered K pair, one 128-partition matmul
                        p, li = u[1], u[4] - k0
                        nc.tensor.matmul(
                            st[:, c0: c0 + BLK],
                            lhsT=kp[:, li * 128: (li + 1) * 128],
                            rhs=qT[:, p * BLK: (p + 1) * BLK],
                            start=True, stop=True, skip_group_check=True)

            ob_tiles = {}
            slots_left = {}

            def finalize(h, bank):
                ob = ob_tiles.pop((h, bank))
                oc = fpool.tile([128, 8 * BLK], BF16, tag="oc",
                                name=f"och{h}_{bank}")
                nc.vector.tensor_copy(oc[0:DA, :], ob[0:DA, :])
                nc.gpsimd.dma_start(out=o_d[h][bank][:, :], in_=oc[0:DA, :])

            def emit_pv(h, wave, pT, vp, s0):
                _, _, v2 = tiles[h]
                for slot, u in wave:
                    if u[0] == "X":
                        continue
                    for (p0, ps, width, src) in _unit_pv(u, slot):
                        bank = POSBANK[p0]
                        key = (h, bank)
                        is_open = u[0] == "G"
                        if is_open:
                            ob = obpool.tile([128, 512], F32, tag="ob",
                                             name=f"obh{h}_{bank}")
                            ob_tiles[key] = ob
                            slots_left[key] = int(
                                sum(pos_chunks[p] for p in BANKS[bank]))
                        elif key not in ob_tiles:
                            # bank 7 has no G only when hpc tail trimmed
                            raise AssertionError("bank not opened")
                        ob = ob_tiles[key]
                        if src[0] == "v2":
                            lhsT = v2[:, src[1] * DA: (src[1] + 1) * DA]
                        else:
                            li = src[1] - s0
                            lhsT = vp[:, li * DA: (li + 1) * DA]
                        col0 = (p0 - BANKS[bank].start) * BLK
                        slots_left[key] -= width
                        nc.tensor.matmul(
                            ob[0:DA, col0: col0 + width * BLK],
                            lhsT=lhsT,
                            rhs=pT[:, ps * BLK: (ps + width) * BLK],
                            start=is_open, stop=(slots_left[key] == 0),
                            skip_group_check=True)
                        if slots_left[key] == 0:
                            del slots_left[key]
                            finalize(h, bank)

            load_head(0, part=1)
            WP = (W + GW - 1) // GW
            for gp in range(VP_LOOKAHEAD // GW):
                load_gather(gp)
            load_head(0, part=2)

            pend = {}
            total = hpc * W
            for g in range(total + PV_LAG):
                if g < total:
                    h, w = divmod(g, W)
                    wave = waves[w]
                    gpair = h * WP + w // GW
                    st = stpool.tile([128, WAVE_CHUNKS * BLK], F32,
                                     tag="st", name=f"sth{h}_{w}")
                    emit_qk(h, wave, st, kps.get(gpair),
                            pair_s0.get(gpair, 0))
                    pT = ppool.tile([128, WAVE_CHUNKS * BLK], BF16,
                                    tag="pT", name=f"pTh{h}_{w}")
                    ncols = (wave[-1][0] + _usize(wave[-1][1])) * BLK
                    nc.scalar.activation(
                        out=pT[:, :ncols], in_=st[:, :ncols],
                        func=mybir.ActivationFunctionType.Exp, scale=SCALE)
                    pend[g] = (h, wave, pT)
                    if w == 6 and h + 1 < hpc:
                        load_head(h + 1)
                    gl = g + VP_LOOKAHEAD
                    if gl % GW == 0:
                        hl, wl = divmod(gl, W)
                        load_gather(hl * WP + wl // GW)
                gpv = g - PV_LAG
                if gpv >= 0:
                    h2, w2 = divmod(gpv, W)
                    gpair2 = h2 * WP + w2 // GW
                    _, wave2, pT2 = pend.pop(gpv)
                    emit_pv(h2, wave2, pT2, vps.get(gpair2),
                            pair_s0.get(gpair2, 0))
    import bass_rust as _bass_rust
    _bass_rust.move_matmul_waits_to_ldweights(nc.m)
    _bass_rust.generate_event_semaphores(nc)
    return nc


_CACHE = {}


def _get_program(bm: np.ndarray):
    key = bm.tobytes()
    if key not in _CACHE:
        _CACHE[key] = _build_program(bm)
    return _CACHE[key]


# -------------------------------------------------------------------- entry

def _prep_inputs(q, k, v, waves, ns):
    import ml_dtypes
    bf16 = ml_dtypes.bfloat16
    q = np.ascontiguousarray(np.asarray(q), dtype=np.float32)
    k = np.ascontiguousarray(np.asarray(k), dtype=np.float32)
    v = np.ascontiguousarray(np.asarray(v), dtype=np.float32)
    qT = q.reshape(B * H, S, D).transpose(0, 2, 1).astype(bf16)
    qTr = qT.reshape(B * H, D, NB, BLK)[:, :, PERM, :].reshape(B * H, D, S)
    qTr = np.concatenate(
        [qTr, np.zeros((B * H, 128 - D, S), dtype=bf16)], axis=1)
    qTr = np.ascontiguousarray(qTr)
    kT = k.reshape(B * H, S, D).transpose(0, 2, 1).astype(bf16)
    kTe = np.ascontiguousarray(np.concatenate(
        [kT, kT[:, :, :BLK], kT[:, :, (NB - 1) * BLK:]], axis=2))
    vA = np.concatenate(
        [v.reshape(B * H, S, D),
         np.ones((B * H, S, 1), dtype=np.float32)], axis=2).astype(bf16)
    v2 = vA.reshape(B * H, NCHUNK, 128, DA)
    vblk = vA.reshape(B * H, NB, BLK, DA)
    pair_g = np.concatenate([vblk[:, 0], vblk[:, NB - 1]], axis=1)
    v2e = np.ascontiguousarray(
        np.concatenate([v2, pair_g[:, None]], axis=1)
        .transpose(0, 2, 1, 3).reshape(B * H, 128, NPAIR * DA))
    kblk = kTe[:, :, :S].reshape(B * H, D, NB, BLK)
    vp = np.zeros((B * H, max(ns, 1), 128, DA), dtype=bf16)
    kp = np.zeros((B * H, max(ns, 1), 64, 128), dtype=bf16)
    for wave in waves:
        for slot, u in wave:
            if u[0] != "S":
                continue
            for half, g in enumerate(u[2:4]):
                if g is not None:
                    vp[:, u[4], half * 64: half * 64 + 64, :] = vblk[:, g]
                    kp[:, u[4], :, half * 64: half * 64 + 64] = \
                        kblk[:, :, g, :]
    vp = np.ascontiguousarray(
        vp.transpose(0, 2, 1, 3).reshape(B * H, 128, -1))
    kp = np.ascontiguousarray(
        kp.transpose(0, 2, 1, 3).reshape(B * H, 64, -1))
    return qTr, kTe, v2e, vp, kp


def _run(inputs, trace=False):
    q, k, v, mask = inputs["q"], inputs["k"], inputs["v"], inputs["mask"]
    bm = _block_mask(mask)
    nc = _get_program(bm)
    waves, ns, _ = _build_schedule(bm)
    qTr, kTe, v2e, vp, kp = _prep_inputs(q, k, v, waves, ns)
    in_maps = []
    for c in range(NCORES):
        sl = slice(c * HPC, (c + 1) * HPC)
        in_maps.append({
            "qT": np.ascontiguousarray(qTr[sl]),
            "kT": np.ascontiguousarray(kTe[sl]),
            "v2": np.ascontiguousarray(v2e[sl]),
            "vp": np.ascontiguousarray(vp[sl]),
            "kp": np.ascontiguousarray(kp[sl]),
        })
    bkr = run_bass_kernel_spmd(nc, in_maps, list(range(NCORES)), trace=trace)
    pieces = []
    inv = np.asarray([QPOS[r2] for r2 in range(NB)])
    for r in bkr.results:
        for hh in range(HPC):
            accT = np.asarray(r[f"o_{hh}"]).astype(np.float32)
            accT = accT.transpose(1, 0, 2).reshape(DA, NB, BLK)
            out_pos = (accT[:D] / accT[D:]).transpose(1, 2, 0)  # [NB,BLK,D]
            pieces.append(out_pos[inv].reshape(S, D))
    out = np.stack(pieces, axis=0).reshape(B, H, S, D).astype(np.float32)
    return out, bkr


def kernel(**inputs):
    out, _ = _run(inputs, trace=False)
    return out
